# revision 12
# baseline (speedup 1.0000x reference)
"""Trainium2 Bass kernel for nn_Attention_21895743275585.

Reference computation (per batch b of 4):
  qkv = w_qkv @ x_flat            # 1x1 conv, x_flat [C=256, N=2304]
  q,k l2-normalized per (head, n) along dim_head=64; SCALE=10
  sim = 10 * qhat^T khat per head; attn = softmax(sim, axis=-1)
  out = attn @ v; final = w_out @ out_inner + b_out

Sharding: 8 cores = (batch b, head-half). Each core handles 4 of the 8 heads
of one batch; host sums the partial output projections (2 halves x 2 head
pairs per batch; bias is fed only to half 0 / pair 0).

On-core layout ([partition, free]):
  q,k "channels-major" [d, n] pairs: tile m in {q01,q23,k01,k23} = [128, N]
  v transposed [n, d] per j-tile (from a separate x^T @ w_v^T matmul) with a
  ones column appended so the E@v matmul also yields softmax denominators.
  sim^T chunk [j, i] = k^T q in PSUM (two heads row-packed via tile_position);
  ACT does exp(PSUM)->SBUF in [128, 1024] instructions (no max subtraction
  needed: |sim|<=10 exactly since q,k are unit vectors).
  1/sqrt and 1/x are computed as exp(-0.5 ln x) / exp(-ln x) -- Ln and Exp
  share one ACT table set (pinned to natural_log_exp_and_others).
  Norm rows live at partition bases {0,32,64,96} of [128, N] tiles (engine
  SBUF APs must start at partition 0/32/64/96); [1,N]->[64,N] partition
  broadcasts bounce through small internal DRAM tensors (DRAM APs allow a
  step-0 partition dim).
"""

import math

import numpy as np

B, C, H, W = 4, 256, 48, 48
HEADS, DIM_HEAD, SCALE = 8, 64, 10.0
INNER = HEADS * DIM_HEAD
N = H * W                      # 2304
NJ = N // 128                  # 18 j-tiles
CHUNKS = [(0, 512), (512, 512), (1024, 512), (1536, 512), (2048, 256)]
EPS = 1e-12

WD_NAME = "bf16"               # working dtype: "bf16" | "f32r" | "f32"

_CACHE = {}


def _pin_act_tables():
    """Force every activation onto the natural_log_exp_and_others set so the
    whole kernel needs exactly one ACT table load (Ln+Exp share that set)."""
    import concourse.bacc as bacc_mod
    if getattr(bacc_mod, "_act_tables_pinned", False):
        return
    orig = bacc_mod.get_activation_tables

    def patched(arch):
        t = orig(arch)
        keep = "natural_log_exp_and_others"
        if keep in t:
            return {name: (funcs if name == keep else set())
                    for name, funcs in t.items()}
        return t

    bacc_mod.get_activation_tables = patched
    bacc_mod._act_tables_pinned = True


def _build(wd_name):
    import concourse.bass as bass
    import concourse.tile as tile
    from concourse import bacc, mybir

    _pin_act_tables()

    F32 = mybir.dt.float32
    F32R = mybir.dt.float32r
    WD = mybir.dt.bfloat16 if wd_name == "bf16" else F32

    def mc(ap):
        # matmul operand cast for the fast-fp32 PE path
        return ap.bitcast(F32R) if wd_name == "f32r" else ap

    Ln = mybir.ActivationFunctionType.Ln
    Exp = mybir.ActivationFunctionType.Exp

    nc = bacc.Bacc("TRN2", target_bir_lowering=False, debug=False,
                   enable_asserts=False, num_devices=8)
    x2 = nc.dram_tensor("x2", [2, 128, N], WD, kind="ExternalInput").ap()
    wqk = nc.dram_tensor("wqk", [2, 128, 512], WD, kind="ExternalInput").ap()
    wvT = nc.dram_tensor("wvT", [2, 128, 256], WD, kind="ExternalInput").ap()
    woT = nc.dram_tensor("woT", [2, 128, 256], WD, kind="ExternalInput").ap()
    bias = nc.dram_tensor("bias", [2, 128, 1], F32, kind="ExternalInput").ap()
    ones8 = nc.dram_tensor("ones8", [128, 9], WD, kind="ExternalInput").ap()
    y = nc.dram_tensor("y", [2, 128, N], F32, kind="ExternalOutput").ap()
    # internal DRAM bounce rows for partition broadcasts
    rsd = nc.dram_tensor("rsd", [8, N], F32, kind="Internal").ap()
    rsdd = nc.dram_tensor("rsdd", [4, N], F32, kind="Internal").ap()

    def bcast_row(dram_row_ap, dst_ap, parts):
        src = bass.AP(tensor=dram_row_ap.tensor, offset=dram_row_ap.offset,
                      ap=[[0, parts]] + list(dram_row_ap.ap))
        nc.sync.dma_start(dst_ap, src)

    # m tile -> norm-row base index a: q01->0, k01->1, q23->2, k23->3
    M_OF = [(0, 0), (2, 1), (1, 2), (3, 3)]

    with tile.TileContext(nc) as tc:
        with tc.tile_pool(name="persist", bufs=1) as P, \
             tc.tile_pool(name="bcast", bufs=2) as RSB, \
             tc.tile_pool(name="sq", bufs=3) as SQ, \
             tc.tile_pool(name="esb", bufs=3) as ESB, \
             tc.tile_pool(name="yst", bufs=3) as YST:

            # ---- load inputs ----
            x_sb = [P.tile([128, N], WD, tag=f"x{c}", name=f"x{c}")
                    for c in range(2)]
            wqk_sb = [P.tile([128, 512], WD, tag=f"wqk{c}", name=f"wqk{c}")
                      for c in range(2)]
            wvT_sb = [P.tile([128, 256], WD, tag=f"wvT{c}", name=f"wvT{c}")
                      for c in range(2)]
            woT_sb = [P.tile([128, 256], WD, tag=f"woT{c}", name=f"woT{c}")
                      for c in range(2)]
            bias_sb = [P.tile([128, 1], F32, tag=f"bias{c}", name=f"bias{c}")
                       for c in range(2)]
            ones8_sb = P.tile([128, 9], WD, tag="ones8", name="ones8")
            for c in range(2):
                nc.sync.dma_start(x_sb[c][:, :], x2[c])
                nc.sync.dma_start(wqk_sb[c][:, :], wqk[c])
                nc.sync.dma_start(wvT_sb[c][:, :], wvT[c])
                nc.sync.dma_start(woT_sb[c][:, :], woT[c])
                nc.sync.dma_start(bias_sb[c][:, :], bias[c])
            nc.sync.dma_start(ones8_sb[:, :], ones8)

            # per-partition Exp bias: ln(SCALE) on q rows (bases 0, 64),
            # 0 on k rows (bases 32, 96)
            biasln = P.tile([128, 1], F32, tag="biasln", name="biasln")
            nc.vector.memset(biasln[0:32, :], math.log(SCALE))
            nc.vector.memset(biasln[32:64, :], 0.0)
            nc.vector.memset(biasln[64:96, :], math.log(SCALE))
            nc.vector.memset(biasln[96:128, :], 0.0)

            qk_sb = [P.tile([128, N], WD, tag=f"qk{m}", name=f"qk{m}")
                     for m in range(4)]
            ss8 = P.tile([128, N], F32, tag="ss8", name="ss8")
            ln8 = P.tile([128, N], F32, tag="ln8", name="ln8")
            rs8 = P.tile([128, N], F32, tag="rs8", name="rs8")
            nc.vector.memset(ss8[:, :], 1.0)
            qhat = [P.tile([128, N], WD, tag=f"qh{p}", name=f"qh{p}")
                    for p in range(2)]
            khat = [P.tile([128, N], WD, tag=f"kh{p}", name=f"kh{p}")
                    for p in range(2)]
            vT_sb = P.tile([128, NJ, 4, 64], WD, tag="vT", name="vT")

            numer = [P.tile([128, N], WD, tag=f"nu{p}", name=f"nu{p}")
                     for p in range(2)]
            nsc = [P.tile([128, N], WD, tag=f"nsc{p}", name=f"nsc{p}")
                   for p in range(2)]
            s8 = P.tile([128, N], F32, tag="s8", name="s8")
            lnd8 = P.tile([128, N], F32, tag="lnd8", name="lnd8")
            rsden8 = P.tile([128, N], F32, tag="rsden8", name="rsden8")
            nc.vector.memset(s8[:, :], 1.0)

            # ---- phase 1: QKV projection, norms, v^T ----
            with tc.tile_pool(name="psq", bufs=2, space="PSUM") as PSQ, \
                 tc.tile_pool(name="pss", bufs=2, space="PSUM") as PSS, \
                 tc.tile_pool(name="psv", bufs=2, space="PSUM") as PSV:

                def qkv_m(m, a):
                    base = 32 * a
                    for (off, cw) in CHUNKS:
                        pq = PSQ.tile([128, 512], F32, tag="pq", name="pq")
                        for c in range(2):
                            nc.tensor.matmul(
                                pq[:, 0:cw],
                                mc(wqk_sb[c][:, m * 128:(m + 1) * 128]),
                                mc(x_sb[c][:, off:off + cw]),
                                start=(c == 0), stop=(c == 1))
                        nc.vector.tensor_copy(qk_sb[m][:, off:off + cw],
                                              pq[:, 0:cw])
                        q2 = SQ.tile([128, 512], WD, tag="q2", name="q2")
                        nc.vector.tensor_mul(q2[:, 0:cw],
                                             qk_sb[m][:, off:off + cw],
                                             qk_sb[m][:, off:off + cw])
                        pss = PSS.tile([8, 512], F32, tag="pss", name="pss")
                        nc.tensor.matmul(pss[:, 0:cw], mc(ones8_sb[:, 0:8]),
                                         mc(q2[:, 0:cw]),
                                         start=True, stop=True)
                        nc.vector.tensor_copy(
                            ss8[base:base + 2, off:off + cw],
                            pss[0:2, 0:cw])

                def rs_pair(p):
                    # pair p: q rows at base 64p, k rows at base 64p+32
                    b0 = 64 * p
                    sl = slice(b0, b0 + 64)
                    nc.vector.tensor_scalar_max(ss8[sl, :], ss8[sl, :],
                                                EPS * EPS)
                    nc.scalar.activation(ln8[sl, :], ss8[sl, :], Ln)
                    nc.scalar.activation(rs8[sl, :], ln8[sl, :], Exp,
                                         scale=-0.5, bias=biasln[sl, :])
                    for a in (2 * p, 2 * p + 1):
                        nc.sync.dma_start(rsd[2 * a:2 * a + 2, :],
                                          rs8[32 * a:32 * a + 2, :])

                def normalize_pair(p):
                    for (dst, a, src_m) in ((qhat[p], 2 * p, p),
                                            (khat[p], 2 * p + 1, 2 + p)):
                        rsb = RSB.tile([128, N], F32, tag="rsb", name="rsb")
                        for (off, cw) in CHUNKS:
                            bcast_row(rsd[2 * a][off:off + cw],
                                      rsb[0:64, off:off + cw], 64)
                            bcast_row(rsd[2 * a + 1][off:off + cw],
                                      rsb[64:128, off:off + cw], 64)
                            nc.vector.tensor_mul(
                                dst[:, off:off + cw],
                                qk_sb[src_m][:, off:off + cw],
                                rsb[:, off:off + cw])

                # pair 0 QKV + norms
                qkv_m(0, 0)
                qkv_m(2, 1)
                rs_pair(0)
                normalize_pair(0)

                # v^T via x^T @ w_v^T (overlaps the pair-0 norm chain)
                for jt in range(NJ):
                    pv = PSV.tile([128, 256], F32, tag="pv", name="pv")
                    for c in range(2):
                        nc.tensor.matmul(
                            pv[:, :],
                            mc(x_sb[c][:, jt * 128:(jt + 1) * 128]),
                            mc(wvT_sb[c][:, :]),
                            start=(c == 0), stop=(c == 1))
                    nc.vector.tensor_copy(
                        vT_sb[:, jt, :, :],
                        pv.rearrange("p (h d) -> p h d", h=4))

                # pair 1 QKV + norms
                qkv_m(1, 2)
                qkv_m(3, 3)
                rs_pair(1)
                normalize_pair(1)

            # ---- phase 2+3: attention, scaling, output projection ----
            with tc.tile_pool(name="psring", bufs=1, space="PSUM") as PSR, \
                 tc.tile_pool(name="pso", bufs=1, space="PSUM") as PSO:
                ring = PSR.tile([128, 6, 512], F32, tag="ring", name="ring")

                def attention_pair(hp):
                    for (off, cw) in CHUNKS:
                        po = PSO.tile([128, 512], F32, tag="po", name="po")
                        po_o = PSO.tile([33, 512], F32, tag="po_o",
                                        name="po_o")
                        state = {"sim": 0, "j": 0}
                        e_of_slot = {}

                        def emit_sims_upto(lim):
                            while state["sim"] < min(lim, 2 * NJ):
                                sl = state["sim"]
                                j, h = sl // 2, sl % 2
                                js = slice(j * 128, (j + 1) * 128)
                                nc.tensor.matmul(
                                    ring[:, sl % 6, 0:cw],
                                    mc(khat[hp][64 * h:64 * h + 64, js]),
                                    mc(qhat[hp][64 * h:64 * h + 64,
                                                off:off + cw]),
                                    start=True, stop=True,
                                    tile_position=(64 * h, 0),
                                    skip_group_check=True)
                                state["sim"] += 1

                        def emit_ev_upto(jlim):
                            while state["j"] < jlim:
                                j = state["j"]
                                ea, ia = e_of_slot.pop(2 * j)
                                eb, ib = e_of_slot.pop(2 * j + 1)
                                st, sp = (j == 0), (j == NJ - 1)
                                nc.tensor.matmul(
                                    po[0:64, 0:cw],
                                    mc(vT_sb[:, j, 2 * hp, :]),
                                    mc(ea[:, 512 * ia:512 * ia + cw]),
                                    start=st, stop=sp, tile_position=(0, 0),
                                    skip_group_check=True)
                                nc.tensor.matmul(
                                    po[64:128, 0:cw],
                                    mc(vT_sb[:, j, 2 * hp + 1, :]),
                                    mc(eb[:, 512 * ib:512 * ib + cw]),
                                    start=st, stop=sp, tile_position=(0, 64),
                                    skip_group_check=True)
                                nc.tensor.matmul(
                                    po_o[0:1, 0:cw],
                                    mc(ones8_sb[:, 8:9]),
                                    mc(ea[:, 512 * ia:512 * ia + cw]),
                                    start=st, stop=sp, tile_position=(0, 0),
                                    skip_group_check=True)
                                nc.tensor.matmul(
                                    po_o[32:33, 0:cw],
                                    mc(ones8_sb[:, 8:9]),
                                    mc(eb[:, 512 * ib:512 * ib + cw]),
                                    start=st, stop=sp, tile_position=(0, 32),
                                    skip_group_check=True)
                                state["j"] += 1

                        for k in range(12):  # 12 exps x 3 slots = 36 slots
                            emit_sims_upto(3 * k + 3)
                            base = (3 * k) % 6
                            e = ESB.tile([128, 3, 512], WD, tag="e",
                                         name="e")
                            nc.scalar.activation(
                                e[:, :, 0:cw],
                                ring[:, base:base + 3, 0:cw],
                                Exp)
                            for i in range(3):
                                e_of_slot[3 * k + i] = (
                                    e.rearrange("p a b -> p (a b)"), i)
                            emit_sims_upto(3 * k + 6)
                            # only consume e-tiles one exp generation old so
                            # the in-order PE stream never stalls on a
                            # just-issued exp
                            emit_ev_upto(min((3 * k) // 2, NJ))
                        emit_ev_upto(NJ)
                        # drain numerators + denominators (s rows at base 32h)
                        nc.vector.tensor_copy(numer[hp][:, off:off + cw],
                                              po[:, 0:cw])
                        for t in range(2):
                            h = 2 * hp + t
                            nc.vector.tensor_copy(
                                s8[32 * h:32 * h + 1, off:off + cw],
                                po_o[32 * t:32 * t + 1, 0:cw])

                attention_pair(0)
                attention_pair(1)

            # ---- tail: 1/s (both pairs batched), rescale, output proj ----
            with tc.tile_pool(name="psf", bufs=2, space="PSUM") as PSF:
                nc.scalar.activation(lnd8[:, :], s8[:, :], Ln)
                nc.scalar.activation(rsden8[:, :], lnd8[:, :], Exp,
                                     scale=-1.0)
                for h in range(4):
                    nc.sync.dma_start(rsdd[h:h + 1, :],
                                      rsden8[32 * h:32 * h + 1, :])
                rsb_d = [RSB.tile([128, N], F32, tag=f"rsbd{p}",
                                  name=f"rsbd{p}") for p in range(2)]
                for (off, cw) in CHUNKS:
                    for p in range(2):
                        bcast_row(rsdd[2 * p][off:off + cw],
                                  rsb_d[p][0:64, off:off + cw], 64)
                        bcast_row(rsdd[2 * p + 1][off:off + cw],
                                  rsb_d[p][64:128, off:off + cw], 64)
                        nc.vector.tensor_mul(nsc[p][:, off:off + cw],
                                             numer[p][:, off:off + cw],
                                             rsb_d[p][:, off:off + cw])
                    for m2 in range(2):
                        pf = PSF.tile([128, 512], F32, tag="pf", name="pf")
                        for p in range(2):
                            nc.tensor.matmul(
                                pf[:, 0:cw],
                                mc(woT_sb[p][:, m2 * 128:(m2 + 1) * 128]),
                                mc(nsc[p][:, off:off + cw]),
                                start=(p == 0), stop=(p == 1))
                        yt = YST.tile([128, 512], F32, tag="yt", name="yt")
                        nc.vector.tensor_scalar_add(yt[:, 0:cw], pf[:, 0:cw],
                                                    bias_sb[m2][:, :])
                        nc.sync.dma_start(y[m2][:, off:off + cw], yt[:, 0:cw])

    nc.compile()
    return nc


def _get_program(wd_name=WD_NAME):
    if wd_name not in _CACHE:
        _CACHE[wd_name] = _build(wd_name)
    return _CACHE[wd_name]


def _np_wd(wd_name):
    if wd_name == "bf16":
        import ml_dtypes
        return np.dtype(ml_dtypes.bfloat16)
    return np.dtype(np.float32)


def make_in_maps(x, w_qkv, w_out, b_out, wd_name=WD_NAME):
    x = np.asarray(x, np.float32)
    w_qkv = np.asarray(w_qkv, np.float32)
    w_out = np.asarray(w_out, np.float32)
    b_out = np.asarray(b_out, np.float32)
    wd = _np_wd(wd_name)

    ones8 = np.zeros((128, 9), np.float32)
    ones8[:, 8] = 1.0
    for cc in range(8):
        lo = 64 * (cc % 2)
        ones8[lo:lo + 64, cc] = 1.0

    in_maps = []
    for core in range(8):
        b, half = core // 2, core % 2
        hsel = slice(256 * half, 256 * (half + 1))
        q_rows = np.arange(0, 512)[hsel]
        k_rows = 512 + q_rows
        v_rows = 1024 + q_rows
        wqk_h = np.ascontiguousarray(
            w_qkv[np.r_[q_rows, k_rows], :].T).reshape(2, 128, 512)
        wvT_h = np.ascontiguousarray(w_qkv[v_rows, :].T).reshape(2, 128, 256)
        woT_h = np.ascontiguousarray(w_out[:, hsel].T).reshape(2, 128, 256)
        bias_h = (b_out if half == 0 else np.zeros_like(b_out))
        in_maps.append({
            "x2": x[b].reshape(C, N).reshape(2, 128, N).astype(wd),
            "wqk": wqk_h.astype(wd),
            "wvT": wvT_h.astype(wd),
            "woT": woT_h.astype(wd),
            "bias": bias_h.reshape(2, 128, 1).astype(np.float32),
            "ones8": ones8.astype(wd),
        })
    return in_maps


def gather_output(results):
    outs = [r["y"].reshape(C, N) for r in results]
    return np.stack([
        (outs[2 * b] + outs[2 * b + 1]).reshape(C, H, W) for b in range(B)
    ]).astype(np.float32)


def run(in_maps, wd_name=WD_NAME, **kwargs):
    from concourse import bass_utils
    nc = _get_program(wd_name)
    return bass_utils.run_bass_kernel_spmd(nc, in_maps,
                                           core_ids=list(range(8)), **kwargs)


def kernel(x, w_qkv, w_out, b_out):
    in_maps = make_in_maps(x, w_qkv, w_out, b_out)
    res = run(in_maps)
    return gather_output(res.results)


# revision 13
# speedup vs baseline: 1.0159x; 1.0159x over previous
"""Trainium2 Bass kernel for nn_Attention_21895743275585.

Reference computation (per batch b of 4):
  qkv = w_qkv @ x_flat            # 1x1 conv, x_flat [C=256, N=2304]
  q,k l2-normalized per (head, n) along dim_head=64; SCALE=10
  sim = 10 * qhat^T khat per head; attn = softmax(sim, axis=-1)
  out = attn @ v; final = w_out @ out_inner + b_out

Sharding: 8 cores = (batch b, head-half). Each core handles 4 of the 8 heads
of one batch; host sums the partial output projections (2 halves x 2 head
pairs per batch; bias is fed only to half 0 / pair 0).

On-core layout ([partition, free]):
  q,k "channels-major" [d, n] pairs: tile m in {q01,q23,k01,k23} = [128, N]
  v transposed [n, d] per j-tile (from a separate x^T @ w_v^T matmul) with a
  ones column appended so the E@v matmul also yields softmax denominators.
  sim^T chunk [j, i] = k^T q in PSUM (two heads row-packed via tile_position);
  ACT does exp(PSUM)->SBUF in [128, 1024] instructions (no max subtraction
  needed: |sim|<=10 exactly since q,k are unit vectors).
  1/sqrt and 1/x are computed as exp(-0.5 ln x) / exp(-ln x) -- Ln and Exp
  share one ACT table set (pinned to natural_log_exp_and_others).
  Norm rows live at partition bases {0,32,64,96} of [128, N] tiles (engine
  SBUF APs must start at partition 0/32/64/96); [1,N]->[64,N] partition
  broadcasts bounce through small internal DRAM tensors (DRAM APs allow a
  step-0 partition dim).
"""

import math

import numpy as np

B, C, H, W = 4, 256, 48, 48
HEADS, DIM_HEAD, SCALE = 8, 64, 10.0
INNER = HEADS * DIM_HEAD
N = H * W                      # 2304
NJ = N // 128                  # 18 j-tiles
CHUNKS = [(0, 512), (512, 512), (1024, 512), (1536, 512), (2048, 256)]
EPS = 1e-12

WD_NAME = "bf16"               # working dtype: "bf16" | "f32r" | "f32"

_CACHE = {}


def _pin_act_tables():
    """Force every activation onto the natural_log_exp_and_others set so the
    whole kernel needs exactly one ACT table load (Ln+Exp share that set)."""
    import concourse.bacc as bacc_mod
    if getattr(bacc_mod, "_act_tables_pinned", False):
        return
    orig = bacc_mod.get_activation_tables

    def patched(arch):
        t = orig(arch)
        keep = "natural_log_exp_and_others"
        if keep in t:
            return {name: (funcs if name == keep else set())
                    for name, funcs in t.items()}
        return t

    bacc_mod.get_activation_tables = patched
    bacc_mod._act_tables_pinned = True


def _build(wd_name):
    import concourse.bass as bass
    import concourse.tile as tile
    from concourse import bacc, mybir

    _pin_act_tables()

    F32 = mybir.dt.float32
    F32R = mybir.dt.float32r
    WD = mybir.dt.bfloat16 if wd_name == "bf16" else F32

    def mc(ap):
        # matmul operand cast for the fast-fp32 PE path
        return ap.bitcast(F32R) if wd_name == "f32r" else ap

    Ln = mybir.ActivationFunctionType.Ln
    Exp = mybir.ActivationFunctionType.Exp

    nc = bacc.Bacc("TRN2", target_bir_lowering=False, debug=False,
                   enable_asserts=False, num_devices=8)
    x2 = nc.dram_tensor("x2", [2, 128, N], WD, kind="ExternalInput").ap()
    wqk = nc.dram_tensor("wqk", [2, 128, 512], WD, kind="ExternalInput").ap()
    wvT = nc.dram_tensor("wvT", [2, 128, 256], WD, kind="ExternalInput").ap()
    woT = nc.dram_tensor("woT", [2, 128, 256], WD, kind="ExternalInput").ap()
    bias = nc.dram_tensor("bias", [2, 128, 1], F32, kind="ExternalInput").ap()
    ones8 = nc.dram_tensor("ones8", [128, 9], WD, kind="ExternalInput").ap()
    y = nc.dram_tensor("y", [2, 128, N], F32, kind="ExternalOutput").ap()
    # internal DRAM bounce rows for partition broadcasts
    rsd = nc.dram_tensor("rsd", [8, N], F32, kind="Internal").ap()
    rsdd = nc.dram_tensor("rsdd", [4, N], F32, kind="Internal").ap()

    def bcast_row(dram_row_ap, dst_ap, parts):
        src = bass.AP(tensor=dram_row_ap.tensor, offset=dram_row_ap.offset,
                      ap=[[0, parts]] + list(dram_row_ap.ap))
        nc.sync.dma_start(dst_ap, src)

    # m tile -> norm-row base index a: q01->0, k01->1, q23->2, k23->3
    M_OF = [(0, 0), (2, 1), (1, 2), (3, 3)]

    with tile.TileContext(nc) as tc:
        with tc.tile_pool(name="persist", bufs=1) as P, \
             tc.tile_pool(name="bcast", bufs=2) as RSB, \
             tc.tile_pool(name="sq", bufs=3) as SQ, \
             tc.tile_pool(name="esb", bufs=3) as ESB, \
             tc.tile_pool(name="yst", bufs=3) as YST:

            # ---- load inputs ----
            x_sb = [P.tile([128, N], WD, tag=f"x{c}", name=f"x{c}")
                    for c in range(2)]
            wqk_sb = [P.tile([128, 512], WD, tag=f"wqk{c}", name=f"wqk{c}")
                      for c in range(2)]
            wvT_sb = [P.tile([128, 256], WD, tag=f"wvT{c}", name=f"wvT{c}")
                      for c in range(2)]
            woT_sb = [P.tile([128, 256], WD, tag=f"woT{c}", name=f"woT{c}")
                      for c in range(2)]
            bias_sb = [P.tile([128, 1], F32, tag=f"bias{c}", name=f"bias{c}")
                       for c in range(2)]
            ones8_sb = P.tile([128, 9], WD, tag="ones8", name="ones8")
            for c in range(2):
                nc.sync.dma_start(x_sb[c][:, :], x2[c])
                nc.sync.dma_start(wqk_sb[c][:, :], wqk[c])
                nc.sync.dma_start(wvT_sb[c][:, :], wvT[c])
                nc.sync.dma_start(woT_sb[c][:, :], woT[c])
                nc.sync.dma_start(bias_sb[c][:, :], bias[c])
            nc.sync.dma_start(ones8_sb[:, :], ones8)

            # per-partition Exp bias: ln(SCALE) on q rows (bases 0, 64),
            # 0 on k rows (bases 32, 96)
            biasln = P.tile([128, 1], F32, tag="biasln", name="biasln")
            nc.vector.memset(biasln[0:32, :], math.log(SCALE))
            nc.vector.memset(biasln[32:64, :], 0.0)
            nc.vector.memset(biasln[64:96, :], math.log(SCALE))
            nc.vector.memset(biasln[96:128, :], 0.0)

            qk_sb = [P.tile([128, N], WD, tag=f"qk{m}", name=f"qk{m}")
                     for m in range(4)]
            ss8 = P.tile([128, N], F32, tag="ss8", name="ss8")
            ln8 = P.tile([128, N], F32, tag="ln8", name="ln8")
            rs8 = P.tile([128, N], F32, tag="rs8", name="rs8")
            nc.vector.memset(ss8[:, :], 1.0)
            qhat = [P.tile([128, N], WD, tag=f"qh{p}", name=f"qh{p}")
                    for p in range(2)]
            khat = [P.tile([128, N], WD, tag=f"kh{p}", name=f"kh{p}")
                    for p in range(2)]
            vT_sb = P.tile([128, NJ, 4, 65], WD, tag="vT", name="vT")
            nc.vector.memset(vT_sb[:, :, :, 64:65], 1.0)

            numer = [P.tile([128, N], WD, tag=f"nu{p}", name=f"nu{p}")
                     for p in range(2)]
            nsc = [P.tile([128, N], WD, tag=f"nsc{p}", name=f"nsc{p}")
                   for p in range(2)]
            s8 = P.tile([128, N], F32, tag="s8", name="s8")
            lnd8 = P.tile([128, N], F32, tag="lnd8", name="lnd8")
            rsden8 = P.tile([128, N], F32, tag="rsden8", name="rsden8")
            nc.vector.memset(s8[:, :], 1.0)

            # ---- phase 1: QKV projection, norms, v^T ----
            with tc.tile_pool(name="psq", bufs=2, space="PSUM") as PSQ, \
                 tc.tile_pool(name="pss", bufs=2, space="PSUM") as PSS, \
                 tc.tile_pool(name="psv", bufs=2, space="PSUM") as PSV:

                def qkv_m(m, a):
                    base = 32 * a
                    for (off, cw) in CHUNKS:
                        pq = PSQ.tile([128, 512], F32, tag="pq", name="pq")
                        for c in range(2):
                            nc.tensor.matmul(
                                pq[:, 0:cw],
                                mc(wqk_sb[c][:, m * 128:(m + 1) * 128]),
                                mc(x_sb[c][:, off:off + cw]),
                                start=(c == 0), stop=(c == 1))
                        nc.vector.tensor_copy(qk_sb[m][:, off:off + cw],
                                              pq[:, 0:cw])
                        q2 = SQ.tile([128, 512], WD, tag="q2", name="q2")
                        nc.vector.tensor_mul(q2[:, 0:cw],
                                             qk_sb[m][:, off:off + cw],
                                             qk_sb[m][:, off:off + cw])
                        pss = PSS.tile([8, 512], F32, tag="pss", name="pss")
                        nc.tensor.matmul(pss[:, 0:cw], mc(ones8_sb[:, 0:8]),
                                         mc(q2[:, 0:cw]),
                                         start=True, stop=True)
                        nc.vector.tensor_copy(
                            ss8[base:base + 2, off:off + cw],
                            pss[0:2, 0:cw])

                def rs_pair(p):
                    # pair p: q rows at base 64p, k rows at base 64p+32
                    b0 = 64 * p
                    sl = slice(b0, b0 + 64)
                    nc.vector.tensor_scalar_max(ss8[sl, :], ss8[sl, :],
                                                EPS * EPS)
                    nc.scalar.activation(ln8[sl, :], ss8[sl, :], Ln)
                    nc.scalar.activation(rs8[sl, :], ln8[sl, :], Exp,
                                         scale=-0.5, bias=biasln[sl, :])
                    for a in (2 * p, 2 * p + 1):
                        nc.sync.dma_start(rsd[2 * a:2 * a + 2, :],
                                          rs8[32 * a:32 * a + 2, :])

                def normalize_pair(p):
                    for (dst, a, src_m) in ((qhat[p], 2 * p, p),
                                            (khat[p], 2 * p + 1, 2 + p)):
                        rsb = RSB.tile([128, N], F32, tag="rsb", name="rsb")
                        for (off, cw) in CHUNKS:
                            bcast_row(rsd[2 * a][off:off + cw],
                                      rsb[0:64, off:off + cw], 64)
                            bcast_row(rsd[2 * a + 1][off:off + cw],
                                      rsb[64:128, off:off + cw], 64)
                            nc.vector.tensor_mul(
                                dst[:, off:off + cw],
                                qk_sb[src_m][:, off:off + cw],
                                rsb[:, off:off + cw])

                # pair 0 QKV + norms
                qkv_m(0, 0)
                qkv_m(2, 1)
                rs_pair(0)
                normalize_pair(0)

                # v^T via x^T @ w_v^T (overlaps the pair-0 norm chain)
                for jt in range(NJ):
                    pv = PSV.tile([128, 256], F32, tag="pv", name="pv")
                    for c in range(2):
                        nc.tensor.matmul(
                            pv[:, :],
                            mc(x_sb[c][:, jt * 128:(jt + 1) * 128]),
                            mc(wvT_sb[c][:, :]),
                            start=(c == 0), stop=(c == 1))
                    nc.vector.tensor_copy(
                        vT_sb[:, jt, :, 0:64],
                        pv.rearrange("p (h d) -> p h d", h=4))

                # pair 1 QKV + norms
                qkv_m(1, 2)
                qkv_m(3, 3)
                rs_pair(1)
                normalize_pair(1)

            # ---- phase 2+3: attention, scaling, output projection ----
            with tc.tile_pool(name="psring", bufs=1, space="PSUM") as PSR, \
                 tc.tile_pool(name="pso", bufs=1, space="PSUM") as PSO:
                ring = PSR.tile([128, 6, 512], F32, tag="ring", name="ring")

                def attention_pair(hp):
                    for (off, cw) in CHUNKS:
                        po_a = PSO.tile([65, 512], F32, tag="po_a",
                                        name="po_a")
                        po_b = PSO.tile([65, 512], F32, tag="po_b",
                                        name="po_b")
                        state = {"sim": 0, "j": 0}
                        e_of_slot = {}

                        def emit_sims_upto(lim):
                            while state["sim"] < min(lim, 2 * NJ):
                                sl = state["sim"]
                                j, h = sl // 2, sl % 2
                                js = slice(j * 128, (j + 1) * 128)
                                nc.tensor.matmul(
                                    ring[:, sl % 6, 0:cw],
                                    mc(khat[hp][64 * h:64 * h + 64, js]),
                                    mc(qhat[hp][64 * h:64 * h + 64,
                                                off:off + cw]),
                                    start=True, stop=True,
                                    tile_position=(64 * h, 0),
                                    skip_group_check=True)
                                state["sim"] += 1

                        def emit_ev_upto(jlim):
                            while state["j"] < jlim:
                                j = state["j"]
                                ea, ia = e_of_slot.pop(2 * j)
                                eb, ib = e_of_slot.pop(2 * j + 1)
                                st, sp = (j == 0), (j == NJ - 1)
                                nc.tensor.matmul(
                                    po_a[:, 0:cw],
                                    mc(vT_sb[:, j, 2 * hp, :]),
                                    mc(ea[:, 512 * ia:512 * ia + cw]),
                                    start=st, stop=sp)
                                nc.tensor.matmul(
                                    po_b[:, 0:cw],
                                    mc(vT_sb[:, j, 2 * hp + 1, :]),
                                    mc(eb[:, 512 * ib:512 * ib + cw]),
                                    start=st, stop=sp)
                                state["j"] += 1

                        for k in range(12):  # 12 exps x 3 slots = 36 slots
                            emit_sims_upto(3 * k + 3)
                            base = (3 * k) % 6
                            e = ESB.tile([128, 3, 512], WD, tag="e",
                                         name="e")
                            nc.scalar.activation(
                                e[:, :, 0:cw],
                                ring[:, base:base + 3, 0:cw],
                                Exp)
                            for i in range(3):
                                e_of_slot[3 * k + i] = (
                                    e.rearrange("p a b -> p (a b)"), i)
                            emit_sims_upto(3 * k + 6)
                            # only consume e-tiles one exp generation old so
                            # the in-order PE stream never stalls on a
                            # just-issued exp
                            emit_ev_upto(min((3 * k) // 2, NJ))
                        emit_ev_upto(NJ)
                        # drain numerators + denominators (s rows at base 32h)
                        nc.vector.tensor_copy(numer[hp][0:64, off:off + cw],
                                              po_a[0:64, 0:cw])
                        nc.vector.tensor_copy(numer[hp][64:128, off:off + cw],
                                              po_b[0:64, 0:cw])
                        for t, po in ((0, po_a), (1, po_b)):
                            h = 2 * hp + t
                            nc.vector.tensor_copy(
                                s8[32 * h:32 * h + 1, off:off + cw],
                                po[64:65, 0:cw])

                attention_pair(0)
                attention_pair(1)

            # ---- tail: 1/s (both pairs batched), rescale, output proj ----
            with tc.tile_pool(name="psf", bufs=2, space="PSUM") as PSF:
                nc.scalar.activation(lnd8[:, :], s8[:, :], Ln)
                nc.scalar.activation(rsden8[:, :], lnd8[:, :], Exp,
                                     scale=-1.0)
                for h in range(4):
                    nc.sync.dma_start(rsdd[h:h + 1, :],
                                      rsden8[32 * h:32 * h + 1, :])
                rsb_d = [RSB.tile([128, N], F32, tag=f"rsbd{p}",
                                  name=f"rsbd{p}") for p in range(2)]
                for (off, cw) in CHUNKS:
                    for p in range(2):
                        bcast_row(rsdd[2 * p][off:off + cw],
                                  rsb_d[p][0:64, off:off + cw], 64)
                        bcast_row(rsdd[2 * p + 1][off:off + cw],
                                  rsb_d[p][64:128, off:off + cw], 64)
                        nc.vector.tensor_mul(nsc[p][:, off:off + cw],
                                             numer[p][:, off:off + cw],
                                             rsb_d[p][:, off:off + cw])
                    for m2 in range(2):
                        pf = PSF.tile([128, 512], F32, tag="pf", name="pf")
                        for p in range(2):
                            nc.tensor.matmul(
                                pf[:, 0:cw],
                                mc(woT_sb[p][:, m2 * 128:(m2 + 1) * 128]),
                                mc(nsc[p][:, off:off + cw]),
                                start=(p == 0), stop=(p == 1))
                        yt = YST.tile([128, 512], F32, tag="yt", name="yt")
                        nc.vector.tensor_scalar_add(yt[:, 0:cw], pf[:, 0:cw],
                                                    bias_sb[m2][:, :])
                        nc.sync.dma_start(y[m2][:, off:off + cw], yt[:, 0:cw])

    nc.compile()
    return nc


def _get_program(wd_name=WD_NAME):
    if wd_name not in _CACHE:
        _CACHE[wd_name] = _build(wd_name)
    return _CACHE[wd_name]


def _np_wd(wd_name):
    if wd_name == "bf16":
        import ml_dtypes
        return np.dtype(ml_dtypes.bfloat16)
    return np.dtype(np.float32)


def make_in_maps(x, w_qkv, w_out, b_out, wd_name=WD_NAME):
    x = np.asarray(x, np.float32)
    w_qkv = np.asarray(w_qkv, np.float32)
    w_out = np.asarray(w_out, np.float32)
    b_out = np.asarray(b_out, np.float32)
    wd = _np_wd(wd_name)

    ones8 = np.zeros((128, 9), np.float32)
    ones8[:, 8] = 1.0
    for cc in range(8):
        lo = 64 * (cc % 2)
        ones8[lo:lo + 64, cc] = 1.0

    in_maps = []
    for core in range(8):
        b, half = core // 2, core % 2
        hsel = slice(256 * half, 256 * (half + 1))
        q_rows = np.arange(0, 512)[hsel]
        k_rows = 512 + q_rows
        v_rows = 1024 + q_rows
        wqk_h = np.ascontiguousarray(
            w_qkv[np.r_[q_rows, k_rows], :].T).reshape(2, 128, 512)
        wvT_h = np.ascontiguousarray(w_qkv[v_rows, :].T).reshape(2, 128, 256)
        woT_h = np.ascontiguousarray(w_out[:, hsel].T).reshape(2, 128, 256)
        bias_h = (b_out if half == 0 else np.zeros_like(b_out))
        in_maps.append({
            "x2": x[b].reshape(C, N).reshape(2, 128, N).astype(wd),
            "wqk": wqk_h.astype(wd),
            "wvT": wvT_h.astype(wd),
            "woT": woT_h.astype(wd),
            "bias": bias_h.reshape(2, 128, 1).astype(np.float32),
            "ones8": ones8.astype(wd),
        })
    return in_maps


def gather_output(results):
    outs = [r["y"].reshape(C, N) for r in results]
    return np.stack([
        (outs[2 * b] + outs[2 * b + 1]).reshape(C, H, W) for b in range(B)
    ]).astype(np.float32)


def run(in_maps, wd_name=WD_NAME, **kwargs):
    from concourse import bass_utils
    nc = _get_program(wd_name)
    return bass_utils.run_bass_kernel_spmd(nc, in_maps,
                                           core_ids=list(range(8)), **kwargs)


def kernel(x, w_qkv, w_out, b_out):
    in_maps = make_in_maps(x, w_qkv, w_out, b_out)
    res = run(in_maps)
    return gather_output(res.results)


# revision 14
# speedup vs baseline: 1.2466x; 1.2271x over previous
"""Trainium2 Bass kernel for nn_Attention_21895743275585.

Reference computation (per batch b of 4):
  qkv = w_qkv @ x_flat            # 1x1 conv, x_flat [C=256, N=2304]
  q,k l2-normalized per (head, n) along dim_head=64; SCALE=10
  sim = 10 * qhat^T khat per head; attn = softmax(sim, axis=-1)
  out = attn @ v; final = w_out @ out_inner + b_out

Sharding: 8 cores = (batch b, head-half). Each core handles 4 of the 8 heads
of one batch; host sums the partial output projections (2 halves x 2 head
pairs per batch; bias is fed only to half 0 / pair 0).

On-core layout ([partition, free]):
  q,k "channels-major" [d, n] pairs: tile m in {q01,q23,k01,k23} = [128, N]
  v transposed [n, d] per j-tile (from a separate x^T @ w_v^T matmul) with a
  ones column appended so the E@v matmul also yields softmax denominators.
  sim^T chunk [j, i] = k^T q in PSUM (two heads row-packed via tile_position);
  ACT does exp(PSUM)->SBUF in [128, 1024] instructions (no max subtraction
  needed: |sim|<=10 exactly since q,k are unit vectors).
  1/sqrt and 1/x are computed as exp(-0.5 ln x) / exp(-ln x) -- Ln and Exp
  share one ACT table set (pinned to natural_log_exp_and_others).
  Norm rows live at partition bases {0,32,64,96} of [128, N] tiles (engine
  SBUF APs must start at partition 0/32/64/96); [1,N]->[64,N] partition
  broadcasts bounce through small internal DRAM tensors (DRAM APs allow a
  step-0 partition dim).
"""

import math

import numpy as np

B, C, H, W = 4, 256, 48, 48
HEADS, DIM_HEAD, SCALE = 8, 64, 10.0
INNER = HEADS * DIM_HEAD
N = H * W                      # 2304
NJ = N // 128                  # 18 j-tiles
CHUNKS = [(0, 512), (512, 512), (1024, 512), (1536, 512), (2048, 256)]
EPS = 1e-12

WD_NAME = "bf16"               # working dtype: "bf16" | "f32r" | "f32"

_CACHE = {}


def _pin_act_tables():
    """Force every activation onto the natural_log_exp_and_others set so the
    whole kernel needs exactly one ACT table load (Ln+Exp share that set)."""
    import concourse.bacc as bacc_mod
    if getattr(bacc_mod, "_act_tables_pinned", False):
        return
    orig = bacc_mod.get_activation_tables

    def patched(arch):
        t = orig(arch)
        keep = "natural_log_exp_and_others"
        if keep in t:
            return {name: (funcs if name == keep else set())
                    for name, funcs in t.items()}
        return t

    bacc_mod.get_activation_tables = patched
    bacc_mod._act_tables_pinned = True


def _build(wd_name):
    import concourse.bass as bass
    import concourse.tile as tile
    from concourse import bacc, mybir

    _pin_act_tables()

    F32 = mybir.dt.float32
    F32R = mybir.dt.float32r
    WD = mybir.dt.bfloat16 if wd_name == "bf16" else F32

    def mc(ap):
        # matmul operand cast for the fast-fp32 PE path
        return ap.bitcast(F32R) if wd_name == "f32r" else ap

    Ln = mybir.ActivationFunctionType.Ln
    Exp = mybir.ActivationFunctionType.Exp

    nc = bacc.Bacc("TRN2", target_bir_lowering=False, debug=False,
                   enable_asserts=False, num_devices=8)
    x2 = nc.dram_tensor("x2", [2, 128, N], WD, kind="ExternalInput").ap()
    wqk = nc.dram_tensor("wqk", [2, 128, 512], WD, kind="ExternalInput").ap()
    wvT = nc.dram_tensor("wvT", [2, 128, 256], WD, kind="ExternalInput").ap()
    woT = nc.dram_tensor("woT", [2, 128, 256], WD, kind="ExternalInput").ap()
    bias = nc.dram_tensor("bias", [2, 128, 1], F32, kind="ExternalInput").ap()
    ones8 = nc.dram_tensor("ones8", [128, 9], WD, kind="ExternalInput").ap()
    # output: per head-pair partial projections, summed on host
    y = nc.dram_tensor("y", [2, 2, 128, N], F32, kind="ExternalOutput").ap()
    # internal DRAM bounce rows for partition broadcasts
    rsd = nc.dram_tensor("rsd", [8, N], F32, kind="Internal").ap()
    rsdd = nc.dram_tensor("rsdd", [4, N], F32, kind="Internal").ap()

    def bcast_row(dram_row_ap, dst_ap, parts):
        src = bass.AP(tensor=dram_row_ap.tensor, offset=dram_row_ap.offset,
                      ap=[[0, parts]] + list(dram_row_ap.ap))
        nc.sync.dma_start(dst_ap, src)

    # m tile -> norm-row base index a: q01->0, k01->1, q23->2, k23->3
    M_OF = [(0, 0), (2, 1), (1, 2), (3, 3)]

    with tile.TileContext(nc) as tc:
        with tc.tile_pool(name="persist", bufs=1) as P, \
             tc.tile_pool(name="bcast", bufs=2) as RSB, \
             tc.tile_pool(name="sq", bufs=3) as SQ, \
             tc.tile_pool(name="esb", bufs=3) as ESB, \
             tc.tile_pool(name="yst", bufs=3) as YST, \
             tc.tile_pool(name="psf", bufs=2, space="PSUM") as PSF:

            # ---- load inputs ----
            x_sb = [P.tile([128, N], WD, tag=f"x{c}", name=f"x{c}")
                    for c in range(2)]
            wqk_sb = [P.tile([128, 512], WD, tag=f"wqk{c}", name=f"wqk{c}")
                      for c in range(2)]
            wvT_sb = [P.tile([128, 256], WD, tag=f"wvT{c}", name=f"wvT{c}")
                      for c in range(2)]
            woT_sb = [P.tile([128, 256], WD, tag=f"woT{c}", name=f"woT{c}")
                      for c in range(2)]
            bias_sb = [P.tile([128, 1], F32, tag=f"bias{c}", name=f"bias{c}")
                       for c in range(2)]
            ones8_sb = P.tile([128, 9], WD, tag="ones8", name="ones8")
            for c in range(2):
                nc.sync.dma_start(x_sb[c][:, :], x2[c])
                nc.sync.dma_start(wqk_sb[c][:, :], wqk[c])
                nc.sync.dma_start(wvT_sb[c][:, :], wvT[c])
                nc.sync.dma_start(woT_sb[c][:, :], woT[c])
                nc.sync.dma_start(bias_sb[c][:, :], bias[c])
            nc.sync.dma_start(ones8_sb[:, :], ones8)

            # per-partition Exp bias: ln(SCALE) on q rows (bases 0, 64),
            # 0 on k rows (bases 32, 96)
            biasln = P.tile([128, 1], F32, tag="biasln", name="biasln")
            nc.vector.memset(biasln[0:32, :], math.log(SCALE))
            nc.vector.memset(biasln[32:64, :], 0.0)
            nc.vector.memset(biasln[64:96, :], math.log(SCALE))
            nc.vector.memset(biasln[96:128, :], 0.0)

            qk_sb = [P.tile([128, N], WD, tag=f"qk{m}", name=f"qk{m}")
                     for m in range(4)]
            ss8 = P.tile([128, N], F32, tag="ss8", name="ss8")
            ln8 = P.tile([128, N], F32, tag="ln8", name="ln8")
            rs8 = P.tile([128, N], F32, tag="rs8", name="rs8")
            nc.vector.memset(ss8[:, :], 1.0)
            qhat = [P.tile([128, N], WD, tag=f"qh{p}", name=f"qh{p}")
                    for p in range(2)]
            khat = [P.tile([128, N], WD, tag=f"kh{p}", name=f"kh{p}")
                    for p in range(2)]
            vT_sb = P.tile([128, NJ, 4, 64], WD, tag="vT", name="vT")

            numer = [P.tile([128, N], WD, tag=f"nu{p}", name=f"nu{p}")
                     for p in range(2)]
            nsc = [P.tile([128, N], WD, tag=f"nsc{p}", name=f"nsc{p}")
                   for p in range(2)]
            s8 = P.tile([128, N], F32, tag="s8", name="s8")
            lnd8 = P.tile([128, N], F32, tag="lnd8", name="lnd8")
            rsden8 = P.tile([128, N], F32, tag="rsden8", name="rsden8")
            nc.vector.memset(s8[:, :], 1.0)

            # ---- phase 1: QKV projection, norms, v^T ----
            with tc.tile_pool(name="psq", bufs=2, space="PSUM") as PSQ, \
                 tc.tile_pool(name="pss", bufs=2, space="PSUM") as PSS, \
                 tc.tile_pool(name="psv", bufs=2, space="PSUM") as PSV:

                def qkv_m(m, a):
                    base = 32 * a
                    for (off, cw) in CHUNKS:
                        pq = PSQ.tile([128, 512], F32, tag="pq", name="pq")
                        for c in range(2):
                            nc.tensor.matmul(
                                pq[:, 0:cw],
                                mc(wqk_sb[c][:, m * 128:(m + 1) * 128]),
                                mc(x_sb[c][:, off:off + cw]),
                                start=(c == 0), stop=(c == 1))
                        nc.vector.tensor_copy(qk_sb[m][:, off:off + cw],
                                              pq[:, 0:cw])
                        q2 = SQ.tile([128, 512], WD, tag="q2", name="q2")
                        nc.vector.tensor_mul(q2[:, 0:cw],
                                             qk_sb[m][:, off:off + cw],
                                             qk_sb[m][:, off:off + cw])
                        pss = PSS.tile([8, 512], F32, tag="pss", name="pss")
                        nc.tensor.matmul(pss[:, 0:cw], mc(ones8_sb[:, 0:8]),
                                         mc(q2[:, 0:cw]),
                                         start=True, stop=True)
                        nc.vector.tensor_copy(
                            ss8[base:base + 2, off:off + cw],
                            pss[0:2, 0:cw])

                def rs_pair(p):
                    # pair p: q rows at base 64p, k rows at base 64p+32
                    b0 = 64 * p
                    sl = slice(b0, b0 + 64)
                    nc.vector.tensor_scalar_max(ss8[sl, :], ss8[sl, :],
                                                EPS * EPS)
                    nc.scalar.activation(ln8[sl, :], ss8[sl, :], Ln)
                    nc.scalar.activation(rs8[sl, :], ln8[sl, :], Exp,
                                         scale=-0.5, bias=biasln[sl, :])
                    for a in (2 * p, 2 * p + 1):
                        nc.sync.dma_start(rsd[2 * a:2 * a + 2, :],
                                          rs8[32 * a:32 * a + 2, :])

                def normalize_pair(p):
                    for (dst, a, src_m) in ((qhat[p], 2 * p, p),
                                            (khat[p], 2 * p + 1, 2 + p)):
                        rsb = RSB.tile([128, N], F32, tag="rsb", name="rsb")
                        for (off, cw) in CHUNKS:
                            bcast_row(rsd[2 * a][off:off + cw],
                                      rsb[0:64, off:off + cw], 64)
                            bcast_row(rsd[2 * a + 1][off:off + cw],
                                      rsb[64:128, off:off + cw], 64)
                            nc.vector.tensor_mul(
                                dst[:, off:off + cw],
                                qk_sb[src_m][:, off:off + cw],
                                rsb[:, off:off + cw])

                # pair 0 QKV + norms
                qkv_m(0, 0)
                qkv_m(2, 1)
                rs_pair(0)
                normalize_pair(0)

                # v^T via x^T @ w_v^T (overlaps the pair-0 norm chain)
                for jt in range(NJ):
                    pv = PSV.tile([128, 256], F32, tag="pv", name="pv")
                    for c in range(2):
                        nc.tensor.matmul(
                            pv[:, :],
                            mc(x_sb[c][:, jt * 128:(jt + 1) * 128]),
                            mc(wvT_sb[c][:, :]),
                            start=(c == 0), stop=(c == 1))
                    nc.vector.tensor_copy(
                        vT_sb[:, jt, :, :],
                        pv.rearrange("p (h d) -> p h d", h=4))

                # pair 1 QKV + norms
                qkv_m(1, 2)
                qkv_m(3, 3)
                rs_pair(1)
                normalize_pair(1)

            # ---- phase 2+3: attention, scaling, output projection ----
            with tc.tile_pool(name="pssim", bufs=2, space="PSUM") as PSSIM, \
                 tc.tile_pool(name="pso", bufs=1, space="PSUM") as PSO:

                def attention_pair(hp):
                    for (off, cw) in CHUNKS:
                        po = PSO.tile([128, 512], F32, tag="po", name="po")
                        po_o = PSO.tile([33, 512], F32, tag="po_o",
                                        name="po_o")

                        def sim_pair(jt):
                            js = slice(jt * 128, (jt + 1) * 128)
                            ps = PSSIM.tile([128, 1024], F32, tag="ps",
                                            name="ps")
                            nc.tensor.matmul(
                                ps[:, 0:cw],
                                mc(khat[hp][0:64, js]),
                                mc(qhat[hp][0:64, off:off + cw]),
                                start=True, stop=True, tile_position=(0, 0))
                            nc.tensor.matmul(
                                ps[:, 512:512 + cw],
                                mc(khat[hp][64:128, js]),
                                mc(qhat[hp][64:128, off:off + cw]),
                                start=True, stop=True, tile_position=(64, 0))
                            return ps

                        ps_cur = sim_pair(0)
                        for jt in range(NJ):
                            e = ESB.tile([128, 1024], WD, tag="e", name="e")
                            ps3 = ps_cur.rearrange("p (b c) -> p b c", b=2)
                            e3 = e.rearrange("p (b c) -> p b c", b=2)
                            nc.scalar.activation(e3[:, :, 0:cw],
                                                 ps3[:, :, 0:cw], Exp)
                            if jt + 1 < NJ:
                                ps_cur = sim_pair(jt + 1)
                            st, sp = (jt == 0), (jt == NJ - 1)
                            nc.tensor.matmul(
                                po[0:64, 0:cw],
                                mc(vT_sb[:, jt, 2 * hp, :]),
                                mc(e[:, 0:cw]),
                                start=st, stop=sp, tile_position=(0, 0),
                                skip_group_check=True)
                            nc.tensor.matmul(
                                po[64:128, 0:cw],
                                mc(vT_sb[:, jt, 2 * hp + 1, :]),
                                mc(e[:, 512:512 + cw]),
                                start=st, stop=sp, tile_position=(0, 64),
                                skip_group_check=True)
                            nc.tensor.matmul(
                                po_o[0:1, 0:cw],
                                mc(ones8_sb[:, 8:9]),
                                mc(e[:, 0:cw]),
                                start=st, stop=sp, tile_position=(0, 0),
                                skip_group_check=True)
                            nc.tensor.matmul(
                                po_o[32:33, 0:cw],
                                mc(ones8_sb[:, 8:9]),
                                mc(e[:, 512:512 + cw]),
                                start=st, stop=sp, tile_position=(0, 32),
                                skip_group_check=True)
                        # drain numerators + denominators (s rows at base 32h)
                        nc.vector.tensor_copy(numer[hp][:, off:off + cw],
                                              po[:, 0:cw])
                        for t in range(2):
                            h = 2 * hp + t
                            nc.vector.tensor_copy(
                                s8[32 * h:32 * h + 1, off:off + cw],
                                po_o[32 * t:32 * t + 1, 0:cw])

                def scale_pair(hp):
                    # 1/s for the two heads of this pair, broadcast, rescale
                    b0 = 64 * hp
                    sl = slice(b0, b0 + 64)
                    nc.scalar.activation(lnd8[sl, :], s8[sl, :], Ln)
                    nc.scalar.activation(rsden8[sl, :], lnd8[sl, :], Exp,
                                         scale=-1.0)
                    for t in range(2):
                        h = 2 * hp + t
                        nc.sync.dma_start(rsdd[h:h + 1, :],
                                          rsden8[32 * h:32 * h + 1, :])
                    rsb = RSB.tile([128, N], F32, tag="rsb", name="rsb")
                    for (off, cw) in CHUNKS:
                        bcast_row(rsdd[2 * hp][off:off + cw],
                                  rsb[0:64, off:off + cw], 64)
                        bcast_row(rsdd[2 * hp + 1][off:off + cw],
                                  rsb[64:128, off:off + cw], 64)
                        nc.vector.tensor_mul(nsc[hp][:, off:off + cw],
                                             numer[hp][:, off:off + cw],
                                             rsb[:, off:off + cw])

                def outproj_pair(pr):
                    for m2 in range(2):
                        for (off, cw) in CHUNKS:
                            pf = PSF.tile([128, 512], F32, tag="pf",
                                          name="pf")
                            nc.tensor.matmul(
                                pf[:, 0:cw],
                                mc(woT_sb[pr][:, m2 * 128:(m2 + 1) * 128]),
                                mc(nsc[pr][:, off:off + cw]),
                                start=True, stop=True)
                            yt = YST.tile([128, 512], F32, tag="yt",
                                          name="yt")
                            if pr == 0:
                                nc.vector.tensor_scalar_add(
                                    yt[:, 0:cw], pf[:, 0:cw],
                                    bias_sb[m2][:, :])
                            else:
                                nc.vector.tensor_copy(yt[:, 0:cw],
                                                      pf[:, 0:cw])
                            nc.sync.dma_start(y[pr][m2][:, off:off + cw],
                                              yt[:, 0:cw])

                attention_pair(0)
                scale_pair(0)
                outproj_pair(0)
                attention_pair(1)
                scale_pair(1)
                outproj_pair(1)

    nc.compile()
    return nc


def _get_program(wd_name=WD_NAME):
    if wd_name not in _CACHE:
        _CACHE[wd_name] = _build(wd_name)
    return _CACHE[wd_name]


def _np_wd(wd_name):
    if wd_name == "bf16":
        import ml_dtypes
        return np.dtype(ml_dtypes.bfloat16)
    return np.dtype(np.float32)


def make_in_maps(x, w_qkv, w_out, b_out, wd_name=WD_NAME):
    x = np.asarray(x, np.float32)
    w_qkv = np.asarray(w_qkv, np.float32)
    w_out = np.asarray(w_out, np.float32)
    b_out = np.asarray(b_out, np.float32)
    wd = _np_wd(wd_name)

    ones8 = np.zeros((128, 9), np.float32)
    ones8[:, 8] = 1.0
    for cc in range(8):
        lo = 64 * (cc % 2)
        ones8[lo:lo + 64, cc] = 1.0

    in_maps = []
    for core in range(8):
        b, half = core // 2, core % 2
        hsel = slice(256 * half, 256 * (half + 1))
        q_rows = np.arange(0, 512)[hsel]
        k_rows = 512 + q_rows
        v_rows = 1024 + q_rows
        wqk_h = np.ascontiguousarray(
            w_qkv[np.r_[q_rows, k_rows], :].T).reshape(2, 128, 512)
        wvT_h = np.ascontiguousarray(w_qkv[v_rows, :].T).reshape(2, 128, 256)
        woT_h = np.ascontiguousarray(w_out[:, hsel].T).reshape(2, 128, 256)
        bias_h = (b_out if half == 0 else np.zeros_like(b_out))
        in_maps.append({
            "x2": x[b].reshape(C, N).reshape(2, 128, N).astype(wd),
            "wqk": wqk_h.astype(wd),
            "wvT": wvT_h.astype(wd),
            "woT": woT_h.astype(wd),
            "bias": bias_h.reshape(2, 128, 1).astype(np.float32),
            "ones8": ones8.astype(wd),
        })
    return in_maps


def gather_output(results):
    outs = [r["y"].sum(axis=0).reshape(C, N) for r in results]
    return np.stack([
        (outs[2 * b] + outs[2 * b + 1]).reshape(C, H, W) for b in range(B)
    ]).astype(np.float32)


def run(in_maps, wd_name=WD_NAME, **kwargs):
    from concourse import bass_utils
    nc = _get_program(wd_name)
    return bass_utils.run_bass_kernel_spmd(nc, in_maps,
                                           core_ids=list(range(8)), **kwargs)


def kernel(x, w_qkv, w_out, b_out):
    in_maps = make_in_maps(x, w_qkv, w_out, b_out)
    res = run(in_maps)
    return gather_output(res.results)


# revision 15
# speedup vs baseline: 1.3155x; 1.0553x over previous
"""Trainium2 Bass kernel for nn_Attention_21895743275585.

Reference computation (per batch b of 4):
  qkv = w_qkv @ x_flat            # 1x1 conv, x_flat [C=256, N=2304]
  q,k l2-normalized per (head, n) along dim_head=64; SCALE=10
  sim = 10 * qhat^T khat per head; attn = softmax(sim, axis=-1)
  out = attn @ v; final = w_out @ out_inner + b_out

Sharding: 8 cores = (batch b, head-half). Each core handles 4 of the 8 heads
of one batch; host sums the partial output projections (2 halves x 2 head
pairs per batch; bias is fed only to half 0 / pair 0).

On-core layout ([partition, free]):
  q,k "channels-major" [d, n] pairs: tile m in {q01,q23,k01,k23} = [128, N]
  v transposed [n, d] per j-tile (from a separate x^T @ w_v^T matmul) with a
  ones column appended so the E@v matmul also yields softmax denominators.
  sim^T chunk [j, i] = k^T q in PSUM (two heads row-packed via tile_position);
  ACT does exp(PSUM)->SBUF in [128, 1024] instructions (no max subtraction
  needed: |sim|<=10 exactly since q,k are unit vectors).
  1/sqrt and 1/x are computed as exp(-0.5 ln x) / exp(-ln x) -- Ln and Exp
  share one ACT table set (pinned to natural_log_exp_and_others).
  Norm rows live at partition bases {0,32,64,96} of [128, N] tiles (engine
  SBUF APs must start at partition 0/32/64/96); [1,N]->[64,N] partition
  broadcasts bounce through small internal DRAM tensors (DRAM APs allow a
  step-0 partition dim).
"""

import math

import numpy as np

B, C, H, W = 4, 256, 48, 48
HEADS, DIM_HEAD, SCALE = 8, 64, 10.0
INNER = HEADS * DIM_HEAD
N = H * W                      # 2304
NJ = N // 128                  # 18 j-tiles
CHUNKS = [(0, 512), (512, 512), (1024, 512), (1536, 512), (2048, 256)]
EPS = 1e-12

WD_NAME = "bf16"               # working dtype: "bf16" | "f32r" | "f32"

_CACHE = {}


def _pin_act_tables():
    """Force every activation onto the natural_log_exp_and_others set so the
    whole kernel needs exactly one ACT table load (Ln+Exp share that set)."""
    import concourse.bacc as bacc_mod
    if getattr(bacc_mod, "_act_tables_pinned", False):
        return
    orig = bacc_mod.get_activation_tables

    def patched(arch):
        t = orig(arch)
        keep = "natural_log_exp_and_others"
        if keep in t:
            return {name: (funcs if name == keep else set())
                    for name, funcs in t.items()}
        return t

    bacc_mod.get_activation_tables = patched
    bacc_mod._act_tables_pinned = True


def _build(wd_name):
    import concourse.bass as bass
    import concourse.tile as tile
    from concourse import bacc, mybir

    _pin_act_tables()

    F32 = mybir.dt.float32
    F32R = mybir.dt.float32r
    WD = mybir.dt.bfloat16 if wd_name == "bf16" else F32

    def mc(ap):
        # matmul operand cast for the fast-fp32 PE path
        return ap.bitcast(F32R) if wd_name == "f32r" else ap

    Ln = mybir.ActivationFunctionType.Ln
    Exp = mybir.ActivationFunctionType.Exp
    ActCopy = mybir.ActivationFunctionType.Copy

    nc = bacc.Bacc("TRN2", target_bir_lowering=False, debug=False,
                   enable_asserts=False, num_devices=8)
    x2 = nc.dram_tensor("x2", [2, 128, N], WD, kind="ExternalInput").ap()
    wqk = nc.dram_tensor("wqk", [2, 128, 512], WD, kind="ExternalInput").ap()
    wvT = nc.dram_tensor("wvT", [2, 128, 256], WD, kind="ExternalInput").ap()
    woT = nc.dram_tensor("woT", [2, 128, 256], WD, kind="ExternalInput").ap()
    bias = nc.dram_tensor("bias", [2, 128, 1], F32, kind="ExternalInput").ap()
    ones8 = nc.dram_tensor("ones8", [128, 9], WD, kind="ExternalInput").ap()
    # output: per head-pair partial projections, summed on host
    y = nc.dram_tensor("y", [2, 2, 128, N], F32, kind="ExternalOutput").ap()
    # internal DRAM bounce rows for partition broadcasts
    rsd = nc.dram_tensor("rsd", [8, N], F32, kind="Internal").ap()
    rsdd = nc.dram_tensor("rsdd", [4, N], F32, kind="Internal").ap()

    def bcast_row(dram_row_ap, dst_ap, parts):
        src = bass.AP(tensor=dram_row_ap.tensor, offset=dram_row_ap.offset,
                      ap=[[0, parts]] + list(dram_row_ap.ap))
        nc.sync.dma_start(dst_ap, src)

    # m tile -> norm-row base index a: q01->0, k01->1, q23->2, k23->3
    M_OF = [(0, 0), (2, 1), (1, 2), (3, 3)]

    with tile.TileContext(nc) as tc:
        with tc.tile_pool(name="persist", bufs=1) as P, \
             tc.tile_pool(name="bcast", bufs=2) as RSB, \
             tc.tile_pool(name="sq", bufs=3) as SQ, \
             tc.tile_pool(name="esb", bufs=3) as ESB, \
             tc.tile_pool(name="yst", bufs=3) as YST, \
             tc.tile_pool(name="psf", bufs=2, space="PSUM") as PSF:

            # ---- load inputs ----
            x_sb = [P.tile([128, N], WD, tag=f"x{c}", name=f"x{c}")
                    for c in range(2)]
            wqk_sb = [P.tile([128, 512], WD, tag=f"wqk{c}", name=f"wqk{c}")
                      for c in range(2)]
            wvT_sb = [P.tile([128, 256], WD, tag=f"wvT{c}", name=f"wvT{c}")
                      for c in range(2)]
            woT_sb = [P.tile([128, 256], WD, tag=f"woT{c}", name=f"woT{c}")
                      for c in range(2)]
            bias_sb = [P.tile([128, 1], F32, tag=f"bias{c}", name=f"bias{c}")
                       for c in range(2)]
            ones8_sb = P.tile([128, 9], WD, tag="ones8", name="ones8")
            for c in range(2):
                nc.sync.dma_start(x_sb[c][:, :], x2[c])
                nc.sync.dma_start(wqk_sb[c][:, :], wqk[c])
                nc.sync.dma_start(wvT_sb[c][:, :], wvT[c])
                nc.sync.dma_start(woT_sb[c][:, :], woT[c])
                nc.sync.dma_start(bias_sb[c][:, :], bias[c])
            nc.sync.dma_start(ones8_sb[:, :], ones8)

            # per-partition Exp bias: ln(SCALE) on q rows (bases 0, 64),
            # 0 on k rows (bases 32, 96)
            biasln = P.tile([128, 1], F32, tag="biasln", name="biasln")
            nc.vector.memset(biasln[0:32, :], math.log(SCALE))
            nc.vector.memset(biasln[32:64, :], 0.0)
            nc.vector.memset(biasln[64:96, :], math.log(SCALE))
            nc.vector.memset(biasln[96:128, :], 0.0)

            qk_sb = [P.tile([128, N], WD, tag=f"qk{m}", name=f"qk{m}")
                     for m in range(4)]
            ss8 = P.tile([128, N], F32, tag="ss8", name="ss8")
            ln8 = P.tile([128, N], F32, tag="ln8", name="ln8")
            rs8 = P.tile([128, N], F32, tag="rs8", name="rs8")
            nc.vector.memset(ss8[:, :], 1.0)
            qhat = [P.tile([128, N], WD, tag=f"qh{p}", name=f"qh{p}")
                    for p in range(2)]
            khat = [P.tile([128, N], WD, tag=f"kh{p}", name=f"kh{p}")
                    for p in range(2)]
            vT_sb = P.tile([128, NJ, 4, 64], WD, tag="vT", name="vT")

            numer = [P.tile([128, N], WD, tag=f"nu{p}", name=f"nu{p}")
                     for p in range(2)]
            nsc = [P.tile([128, N], WD, tag=f"nsc{p}", name=f"nsc{p}")
                   for p in range(2)]
            s8 = P.tile([128, N], F32, tag="s8", name="s8")
            lnd8 = P.tile([128, N], F32, tag="lnd8", name="lnd8")
            rsden8 = P.tile([128, N], F32, tag="rsden8", name="rsden8")
            nc.vector.memset(s8[:, :], 1.0)

            # ---- phase 1: QKV projection, norms, v^T ----
            with tc.tile_pool(name="psq", bufs=2, space="PSUM") as PSQ, \
                 tc.tile_pool(name="pss", bufs=2, space="PSUM") as PSS, \
                 tc.tile_pool(name="psv", bufs=2, space="PSUM") as PSV:

                def qkv_m(m, a):
                    base = 32 * a
                    for (off, cw) in CHUNKS:
                        pq = PSQ.tile([128, 512], F32, tag="pq", name="pq")
                        for c in range(2):
                            nc.tensor.matmul(
                                pq[:, 0:cw],
                                mc(wqk_sb[c][:, m * 128:(m + 1) * 128]),
                                mc(x_sb[c][:, off:off + cw]),
                                start=(c == 0), stop=(c == 1))
                        nc.scalar.activation(qk_sb[m][:, off:off + cw],
                                             pq[:, 0:cw], ActCopy)
                        q2 = SQ.tile([128, 512], WD, tag="q2", name="q2")
                        nc.vector.tensor_mul(q2[:, 0:cw],
                                             qk_sb[m][:, off:off + cw],
                                             qk_sb[m][:, off:off + cw])
                        pss = PSS.tile([8, 512], F32, tag="pss", name="pss")
                        nc.tensor.matmul(pss[:, 0:cw], mc(ones8_sb[:, 0:8]),
                                         mc(q2[:, 0:cw]),
                                         start=True, stop=True)
                        nc.vector.tensor_copy(
                            ss8[base:base + 2, off:off + cw],
                            pss[0:2, 0:cw])

                def rs_pair(p):
                    # pair p: q rows at base 64p, k rows at base 64p+32
                    b0 = 64 * p
                    sl = slice(b0, b0 + 64)
                    nc.vector.tensor_scalar_max(ss8[sl, :], ss8[sl, :],
                                                EPS * EPS)
                    nc.scalar.activation(ln8[sl, :], ss8[sl, :], Ln)
                    nc.scalar.activation(rs8[sl, :], ln8[sl, :], Exp,
                                         scale=-0.5, bias=biasln[sl, :])
                    for a in (2 * p, 2 * p + 1):
                        nc.sync.dma_start(rsd[2 * a:2 * a + 2, :],
                                          rs8[32 * a:32 * a + 2, :])

                def normalize_pair(p):
                    for (dst, a, src_m) in ((qhat[p], 2 * p, p),
                                            (khat[p], 2 * p + 1, 2 + p)):
                        rsb = RSB.tile([128, N], F32, tag="rsb", name="rsb")
                        for (off, cw) in CHUNKS:
                            bcast_row(rsd[2 * a][off:off + cw],
                                      rsb[0:64, off:off + cw], 64)
                            bcast_row(rsd[2 * a + 1][off:off + cw],
                                      rsb[64:128, off:off + cw], 64)
                            nc.vector.tensor_mul(
                                dst[:, off:off + cw],
                                qk_sb[src_m][:, off:off + cw],
                                rsb[:, off:off + cw])

                # pair 0 QKV + norms
                qkv_m(0, 0)
                qkv_m(2, 1)
                rs_pair(0)
                normalize_pair(0)

                # v^T via x^T @ w_v^T (overlaps the pair-0 norm chain)
                for jt in range(NJ):
                    pv = PSV.tile([128, 256], F32, tag="pv", name="pv")
                    for c in range(2):
                        nc.tensor.matmul(
                            pv[:, :],
                            mc(x_sb[c][:, jt * 128:(jt + 1) * 128]),
                            mc(wvT_sb[c][:, :]),
                            start=(c == 0), stop=(c == 1))
                    nc.vector.tensor_copy(
                        vT_sb[:, jt, :, :],
                        pv.rearrange("p (h d) -> p h d", h=4))

                # pair 1 QKV + norms
                qkv_m(1, 2)
                qkv_m(3, 3)
                rs_pair(1)
                normalize_pair(1)

            # ---- phase 2+3: attention, scaling, output projection ----
            with tc.tile_pool(name="pssim", bufs=2, space="PSUM") as PSSIM, \
                 tc.tile_pool(name="pso", bufs=1, space="PSUM") as PSO:

                def attention_pair(hp):
                    for (off, cw) in CHUNKS:
                        po = PSO.tile([128, 512], F32, tag="po", name="po")
                        po_o = PSO.tile([33, 512], F32, tag="po_o",
                                        name="po_o")

                        def sim_pair(jt):
                            js = slice(jt * 128, (jt + 1) * 128)
                            ps = PSSIM.tile([128, 1024], F32, tag="ps",
                                            name="ps")
                            nc.tensor.matmul(
                                ps[:, 0:cw],
                                mc(khat[hp][0:64, js]),
                                mc(qhat[hp][0:64, off:off + cw]),
                                start=True, stop=True, tile_position=(0, 0))
                            nc.tensor.matmul(
                                ps[:, 512:512 + cw],
                                mc(khat[hp][64:128, js]),
                                mc(qhat[hp][64:128, off:off + cw]),
                                start=True, stop=True, tile_position=(64, 0))
                            return ps

                        ps_cur = sim_pair(0)
                        for jt in range(NJ):
                            e = ESB.tile([128, 1024], WD, tag="e", name="e")
                            ps3 = ps_cur.rearrange("p (b c) -> p b c", b=2)
                            e3 = e.rearrange("p (b c) -> p b c", b=2)
                            nc.scalar.activation(e3[:, :, 0:cw],
                                                 ps3[:, :, 0:cw], Exp)
                            if jt + 1 < NJ:
                                ps_cur = sim_pair(jt + 1)
                            st, sp = (jt == 0), (jt == NJ - 1)
                            nc.tensor.matmul(
                                po[0:64, 0:cw],
                                mc(vT_sb[:, jt, 2 * hp, :]),
                                mc(e[:, 0:cw]),
                                start=st, stop=sp, tile_position=(0, 0),
                                skip_group_check=True)
                            nc.tensor.matmul(
                                po[64:128, 0:cw],
                                mc(vT_sb[:, jt, 2 * hp + 1, :]),
                                mc(e[:, 512:512 + cw]),
                                start=st, stop=sp, tile_position=(0, 64),
                                skip_group_check=True)
                            nc.tensor.matmul(
                                po_o[0:1, 0:cw],
                                mc(ones8_sb[:, 8:9]),
                                mc(e[:, 0:cw]),
                                start=st, stop=sp, tile_position=(0, 0),
                                skip_group_check=True)
                            nc.tensor.matmul(
                                po_o[32:33, 0:cw],
                                mc(ones8_sb[:, 8:9]),
                                mc(e[:, 512:512 + cw]),
                                start=st, stop=sp, tile_position=(0, 32),
                                skip_group_check=True)
                        # drain numerators + denominators (s rows at base 32h)
                        nc.vector.tensor_copy(numer[hp][:, off:off + cw],
                                              po[:, 0:cw])
                        for t in range(2):
                            h = 2 * hp + t
                            nc.vector.tensor_copy(
                                s8[32 * h:32 * h + 1, off:off + cw],
                                po_o[32 * t:32 * t + 1, 0:cw])

                def scale_pair(hp):
                    # 1/s for the two heads of this pair, broadcast, rescale
                    b0 = 64 * hp
                    sl = slice(b0, b0 + 64)
                    nc.scalar.activation(lnd8[sl, :], s8[sl, :], Ln)
                    nc.scalar.activation(rsden8[sl, :], lnd8[sl, :], Exp,
                                         scale=-1.0)
                    for t in range(2):
                        h = 2 * hp + t
                        nc.sync.dma_start(rsdd[h:h + 1, :],
                                          rsden8[32 * h:32 * h + 1, :])
                    rsb = RSB.tile([128, N], F32, tag="rsb", name="rsb")
                    for (off, cw) in CHUNKS:
                        bcast_row(rsdd[2 * hp][off:off + cw],
                                  rsb[0:64, off:off + cw], 64)
                        bcast_row(rsdd[2 * hp + 1][off:off + cw],
                                  rsb[64:128, off:off + cw], 64)
                        nc.vector.tensor_mul(nsc[hp][:, off:off + cw],
                                             numer[hp][:, off:off + cw],
                                             rsb[:, off:off + cw])

                def outproj_pair(pr):
                    for m2 in range(2):
                        for (off, cw) in CHUNKS:
                            pf = PSF.tile([128, 512], F32, tag="pf",
                                          name="pf")
                            nc.tensor.matmul(
                                pf[:, 0:cw],
                                mc(woT_sb[pr][:, m2 * 128:(m2 + 1) * 128]),
                                mc(nsc[pr][:, off:off + cw]),
                                start=True, stop=True)
                            yt = YST.tile([128, 512], F32, tag="yt",
                                          name="yt")
                            if pr == 0:
                                nc.vector.tensor_scalar_add(
                                    yt[:, 0:cw], pf[:, 0:cw],
                                    bias_sb[m2][:, :])
                            else:
                                nc.vector.tensor_copy(yt[:, 0:cw],
                                                      pf[:, 0:cw])
                            nc.sync.dma_start(y[pr][m2][:, off:off + cw],
                                              yt[:, 0:cw])

                attention_pair(0)
                scale_pair(0)
                outproj_pair(0)
                attention_pair(1)
                scale_pair(1)
                outproj_pair(1)

    nc.compile()
    return nc


def _get_program(wd_name=WD_NAME):
    if wd_name not in _CACHE:
        _CACHE[wd_name] = _build(wd_name)
    return _CACHE[wd_name]


def _np_wd(wd_name):
    if wd_name == "bf16":
        import ml_dtypes
        return np.dtype(ml_dtypes.bfloat16)
    return np.dtype(np.float32)


def make_in_maps(x, w_qkv, w_out, b_out, wd_name=WD_NAME):
    x = np.asarray(x, np.float32)
    w_qkv = np.asarray(w_qkv, np.float32)
    w_out = np.asarray(w_out, np.float32)
    b_out = np.asarray(b_out, np.float32)
    wd = _np_wd(wd_name)

    ones8 = np.zeros((128, 9), np.float32)
    ones8[:, 8] = 1.0
    for cc in range(8):
        lo = 64 * (cc % 2)
        ones8[lo:lo + 64, cc] = 1.0

    in_maps = []
    for core in range(8):
        b, half = core // 2, core % 2
        hsel = slice(256 * half, 256 * (half + 1))
        q_rows = np.arange(0, 512)[hsel]
        k_rows = 512 + q_rows
        v_rows = 1024 + q_rows
        wqk_h = np.ascontiguousarray(
            w_qkv[np.r_[q_rows, k_rows], :].T).reshape(2, 128, 512)
        wvT_h = np.ascontiguousarray(w_qkv[v_rows, :].T).reshape(2, 128, 256)
        woT_h = np.ascontiguousarray(w_out[:, hsel].T).reshape(2, 128, 256)
        bias_h = (b_out if half == 0 else np.zeros_like(b_out))
        in_maps.append({
            "x2": x[b].reshape(C, N).reshape(2, 128, N).astype(wd),
            "wqk": wqk_h.astype(wd),
            "wvT": wvT_h.astype(wd),
            "woT": woT_h.astype(wd),
            "bias": bias_h.reshape(2, 128, 1).astype(np.float32),
            "ones8": ones8.astype(wd),
        })
    return in_maps


def gather_output(results):
    outs = [r["y"].sum(axis=0).reshape(C, N) for r in results]
    return np.stack([
        (outs[2 * b] + outs[2 * b + 1]).reshape(C, H, W) for b in range(B)
    ]).astype(np.float32)


def run(in_maps, wd_name=WD_NAME, **kwargs):
    from concourse import bass_utils
    nc = _get_program(wd_name)
    return bass_utils.run_bass_kernel_spmd(nc, in_maps,
                                           core_ids=list(range(8)), **kwargs)


def kernel(x, w_qkv, w_out, b_out):
    in_maps = make_in_maps(x, w_qkv, w_out, b_out)
    res = run(in_maps)
    return gather_output(res.results)
